# revision 14
# baseline (speedup 1.0000x reference)
"""VQ-VAE Encoder_conv kernel for 8x Trainium2 NeuronCores (Bass/Tile).

Sharding: data-parallel over batch (8 images -> 8 cores). Each core runs the
full conv stack for one image plus the VQ distance cross-term; the host does
the final argmin (exact f32 replica of the reference arithmetic) and the
trivial emb[idx] gather.

Precision: conv1 uses bf16 hi/lo split matmuls (~2^-17 effective); conv2 and
conv3 default to float32r (~2^-13); conv4/residuals/VQ run in fp32. This
reproduces the reference's VQ argmin indices exactly (including the quantized
tie-break structure of the reference's f32 distance formula).
"""
import sys
sys.path.insert(0, '/opt/trn_rl_repo')

import numpy as np
import ml_dtypes
from contextlib import ExitStack

import concourse.bass as bass
import concourse.mybir as mybir
import concourse.tile as tile
from concourse import bacc
from concourse.bass_utils import run_bass_kernel_spmd

F32 = mybir.dt.float32
F32R = mybir.dt.float32r
BF16 = mybir.dt.bfloat16
BF = ml_dtypes.bfloat16

EPS = 1e-10
BETA = 1.0

# per-layer precision: conv2 in {'f32r','hilo'}; conv3 in {'f32r','fp32','hilo'}
PREC2 = 'f32r'
PREC3 = 'f32r'

# ---------------------------------------------------------------- host prep

def _hilo(a):
    hi = a.astype(BF)
    lo = (a.astype(np.float32) - hi.astype(np.float32)).astype(BF)
    return hi, lo


def _prep_shared(w_in, b_in, w_h1, b_h1, w_h2, b_h2, w_h3, b_h3,
                 r0_w1, r0_w2, r1_w1, r1_w2, emb):
    """Weight layout transforms (shared across all cores)."""
    d = {}
    # conv1: lhsT rows p = dx*12 + dy*3 + i  (must match x im2col partition order)
    w1 = w_in.transpose(3, 2, 1, 0).reshape(48, 128)      # (dx,dy,i) x O
    w1hi, w1lo = _hilo(w1)
    d['w1a'] = np.concatenate([w1hi, w1hi], axis=0)        # [96,128] hi block twice
    d['w1b'] = np.ascontiguousarray(w1lo)                  # [48,128]
    d['b1'] = b_in.reshape(128, 1).astype(np.float32)

    # conv2: [I=128, tap=49, O=256]
    w2 = np.ascontiguousarray(
        w_h1.transpose(2, 3, 1, 0).reshape(49, 128, 256).transpose(1, 0, 2))
    if PREC2 == 'f32r':
        d['w2f'] = w2.astype(np.float32)
    else:
        d['w2hi'], d['w2lo'] = _hilo(w2)
    d['b2'] = b_h1.reshape(2, 128).T.astype(np.float32).copy()   # [128,2]

    # conv3: [128, kc=2, tap=49, O=256]
    w3 = np.ascontiguousarray(
        w_h2.transpose(2, 3, 1, 0).reshape(49, 2, 128, 256).transpose(2, 1, 0, 3))
    if PREC3 in ('f32r', 'fp32'):
        d['w3f'] = w3.astype(np.float32)
    else:
        d['w3hi'], d['w3lo'] = _hilo(w3)
    d['b3'] = b_h2.reshape(2, 128).T.astype(np.float32).copy()

    # conv4: [128, kc=2, tap=9, O=256] f32
    w4 = np.ascontiguousarray(
        w_h3.transpose(2, 3, 1, 0).reshape(9, 2, 128, 256).transpose(2, 1, 0, 3)
    ).astype(np.float32)
    d['b4'] = b_h3.reshape(2, 128).T.astype(np.float32).copy()

    # residual blocks: rw1 [128, kc=2, dx=3, O=1024]; rw2 [128, kc=8, O=256]
    parts = [w4.reshape(128, -1)]
    for bi, (wa, wb) in enumerate(((r0_w1, r0_w2), (r1_w1, r1_w2))):
        rw1 = wa.transpose(2, 3, 1, 0).reshape(3, 2, 128, 1024).transpose(2, 1, 0, 3)
        parts.append(np.ascontiguousarray(rw1).astype(np.float32).reshape(128, -1))
        rw2 = wb[:, :, 0, 0].T.reshape(8, 128, 256).transpose(1, 0, 2)
        parts.append(np.ascontiguousarray(rw2).astype(np.float32).reshape(128, -1))

    # embT: [128, kc=2, 512]
    embt = np.ascontiguousarray(
        emb.T.reshape(2, 128, 512).transpose(1, 0, 2)
    ).astype(np.float32)
    parts.append(embt.reshape(128, -1))
    # wtail: w4(4608) rw1_0(6144) rw2_0(2048) rw1_1(6144) rw2_1(2048) embT(1024)
    d['wtail'] = np.concatenate(parts, axis=1)
    assert d['wtail'].shape == (128, 22016)
    return d


def _prep_x(xb):
    """Per-core im2col of one [3,512,512] image for conv1 (k4 s2 p1).

    Returns [96, 256, 256] bf16: partition p = dx*12 + dy*3 + i holds
    x_pad[i, 2y+dy, 2c+dx]; rows 0-47 = hi part, 48-95 = lo part.
    """
    xp = np.zeros((3, 514, 514), np.float32)
    xp[:, 1:513, 1:513] = xb
    out = np.empty((96, 256, 256), BF)
    for dx in range(4):
        for dy in range(4):
            blk = xp[:, dy:dy + 512:2, dx:dx + 512:2]      # [3,256,256]
            hi, lo = _hilo(blk)
            p = dx * 12 + dy * 3
            out[p:p + 3] = hi
            out[48 + p:48 + p + 3] = lo
    return out


# ------------------------------------------------------------- bass program

_CACHE = {}


def _build(debug=False):
    key = ('prog', debug, PREC2, PREC3)
    if key in _CACHE:
        return _CACHE[key]

    nc = bacc.Bacc("TRN2", target_bir_lowering=False, debug=False, num_devices=8)

    di = {}

    def inp(name, shape, dt):
        di[name] = nc.dram_tensor(name, list(shape), dt, kind="ExternalInput").ap()
        return di[name]

    def outp(name, shape, dt):
        di[name] = nc.dram_tensor(name, list(shape), dt, kind="ExternalOutput").ap()
        return di[name]

    xim = inp('xim', [96, 256, 256], BF16)
    w1a = inp('w1a', [96, 128], BF16)
    w1b = inp('w1b', [48, 128], BF16)
    b1 = inp('b1', [128, 1], F32)
    if PREC2 == 'f32r':
        w2f = inp('w2f', [128, 49, 256], F32)
    else:
        w2hi = inp('w2hi', [128, 49, 256], BF16)
        w2lo = inp('w2lo', [128, 49, 256], BF16)
    b2 = inp('b2', [128, 2], F32)
    if PREC3 in ('f32r', 'fp32'):
        w3f = inp('w3f', [128, 2, 49, 256], F32)
    else:
        w3hi = inp('w3hi', [128, 2, 49, 256], BF16)
        w3lo = inp('w3lo', [128, 2, 49, 256], BF16)
    b3 = inp('b3', [128, 2], F32)
    b4 = inp('b4', [128, 2], F32)
    wtail = inp('wtail', [128, 22016], F32)

    out_lat = outp('out_lat', [2, 128, 64], F32)
    out_twoC = outp('out_twoC', [64, 512], F32)
    if debug:
        h2dt = F32R if PREC2 == 'f32r' else BF16
        outp('dbg_h2hi', [2, 128, 68, 68], h2dt)
        if PREC2 != 'f32r':
            outp('dbg_h2lo', [2, 128, 68, 68], BF16)
        outp('dbg_h3', [2, 128, 18, 18], F32)
        outp('dbg_h4', [2, 128, 8, 10], F32)

    RELU = mybir.ActivationFunctionType.Relu
    COPY = mybir.ActivationFunctionType.Copy
    SUB = mybir.AluOpType.subtract
    MAXOP = mybir.AluOpType.max

    H2DT = F32R if PREC2 == 'f32r' else BF16

    with tile.TileContext(nc) as tc, ExitStack() as ctx:
        pers = ctx.enter_context(tc.tile_pool(name="pers", bufs=1))
        h2p = ctx.enter_context(tc.tile_pool(name="h2p", bufs=1))
        ctx12 = ExitStack()
        w2pool = ctx12.enter_context(tc.tile_pool(name="w2pool", bufs=1))
        # conv2 weights (f32r: DMA f32 in tap chunks, round on DVE; the tmp pool
        # lives alongside the conv1 pools so rounding overlaps conv1 compute)
        w1a_s = pers.tile([96, 128], BF16, tag="w1a")
        nc.gpsimd.dma_start(w1a_s[:], w1a[:])
        w1b_s = pers.tile([48, 128], BF16, tag="w1b")
        nc.gpsimd.dma_start(w1b_s[:], w1b[:])
        b1_s = pers.tile([128, 1], F32, tag="b1")
        nc.gpsimd.dma_start(b1_s[:], b1[:])
        b2_s = pers.tile([128, 2], F32, tag="b2")
        nc.gpsimd.dma_start(b2_s[:], b2[:])

        w2tmp = ctx12.enter_context(tc.tile_pool(name="w2tmp", bufs=1))
        w2r_s = w2pool.tile([128, 49, 256], F32R, tag="w2r")
        for c0 in range(0, 49, 13):
            cn = min(13, 49 - c0)
            w2f_t = w2tmp.tile([128, 13, 256], F32, tag="w2f", name="w2f")
            nc.gpsimd.dma_start(w2f_t[:, :cn, :], w2f[:, c0:c0 + cn])
            nc.vector.tensor_copy(w2r_s[:, c0:c0 + cn], w2f_t[:, :cn, :])

        # h2 (output of conv2, input of conv3), padded 68x68
        h2hi = [h2p.tile([128, 68, 68], H2DT, tag=f"h2hi{k}", name=f"h2hi{k}")
                for k in range(2)]
        h2lo = None
        if PREC2 != 'f32r':
            h2lo = [h2p.tile([128, 68, 68], BF16, tag=f"h2lo{k}", name=f"h2lo{k}")
                    for k in range(2)]
        for k in range(2):
            nc.vector.memset(h2hi[k][:].bitcast(F32) if PREC2 == 'f32r'
                             else h2hi[k][:], 0.0)
            if h2lo is not None:
                nc.vector.memset(h2lo[k][:], 0.0)

        # ---------------- phase 1+2: conv1 (fused strips) + conv2
        with tc.tile_pool(name="xt", bufs=1) as xtp, \
             tc.tile_pool(name="h1", bufs=2) as h1p, \
             tc.tile_pool(name="ps1", bufs=3, space="PSUM") as ps1, \
             tc.tile_pool(name="ps2", bufs=3, space="PSUM") as ps2:

            for s in range(8):
                base = 32 * s - 2                      # h1 row of tile row 0
                y_lo = max(0, base)
                y_hi = min(256, base + 35)
                R = y_hi - y_lo
                t_lo = y_lo - base                     # first valid tile row

                xt = xtp.tile([96, 35, 256], BF16, tag="xt")
                nc.sync.dma_start(xt[0:48, :R, :], xim[0:48, y_lo:y_hi, :])
                nc.scalar.dma_start(xt[48:96, :R, :], xim[48:96, y_lo:y_hi, :])

                if PREC2 == 'f32r':
                    h1t = [h1p.tile([128, 35, 260], F32R, tag="h1r", name="h1r")]
                else:
                    h1t = [h1p.tile([128, 35, 260], BF16, tag="h1hi", name="h1hi"),
                           h1p.tile([128, 35, 260], BF16, tag="h1lo", name="h1lo")]
                for t in h1t:
                    tv = t.bitcast(F32) if PREC2 == 'f32r' else t
                    nc.vector.memset(tv[:, :, 0:2], 0.0)
                    nc.vector.memset(tv[:, :, 258:260], 0.0)
                    if t_lo > 0:
                        nc.vector.memset(tv[:, 0:t_lo, :], 0.0)
                    if t_lo + R < 35:
                        nc.vector.memset(tv[:, t_lo + R:35, :], 0.0)

                # conv1 matmuls: chunks of 2 h1 rows (N=512)
                r = 0
                while r < R:
                    nrow = min(2, R - r)
                    N = nrow * 256
                    ps = ps1.tile([128, 512], F32, tag="c1")
                    rhs = xt[:, r:r + nrow, :]
                    nc.tensor.matmul(ps[:, :N], w1a_s[:], rhs, start=True, stop=False)
                    nc.tensor.matmul(ps[:, :N], w1b_s[:], xt[0:48, r:r + nrow, :],
                                     start=False, stop=True)
                    psv = ps[:, :N].rearrange("p (a b) -> p a b", a=nrow)
                    if PREC2 == 'f32r':
                        dst = h1t[0][:, t_lo + r:t_lo + r + nrow, 2:258]
                        nc.scalar.activation(dst, psv, RELU, bias=b1_s[:])
                    else:
                        raise NotImplementedError("hilo conv2 removed")
                    r += nrow

                # conv2 for this strip: h2 rows [8s, 8s+8)
                for h in range(2):
                    ps = ps2.tile([128, 512], F32, tag="c2")
                    first = True
                    for t in range(49):
                        dy, dx = divmod(t, 7)
                        if PREC2 == 'f32r':
                            rhs = h1t[0][:, dy:dy + 29:4, dx:dx + 253:4]
                            nc.tensor.matmul(ps[:], w2r_s[:, t, 128 * h:128 * h + 128],
                                             rhs, start=first, stop=(t == 48))
                            first = False
                        else:
                            rhs_hi = h1t[0][:, dy:dy + 29:4, dx:dx + 253:4]
                            rhs_lo = h1t[1][:, dy:dy + 29:4, dx:dx + 253:4]
                            lt_hi = w2hi_s[:, t, 128 * h:128 * h + 128]
                            lt_lo = w2lo_s[:, t, 128 * h:128 * h + 128]
                            nc.tensor.matmul(ps[:], lt_hi, rhs_hi, start=first,
                                             stop=False)
                            first = False
                            nc.tensor.matmul(ps[:], lt_hi, rhs_lo, start=False,
                                             stop=False)
                            nc.tensor.matmul(ps[:], lt_lo, rhs_hi, start=False,
                                             stop=(t == 48))
                    psv = ps[:].rearrange("p (a b) -> p a b", a=8)
                    dhi = h2hi[h][:, 2 + 8 * s:10 + 8 * s, 2:66]
                    if PREC2 == 'f32r':
                        nc.scalar.activation(dhi, psv, RELU, bias=b2_s[:, h:h + 1])
                    else:
                        raise NotImplementedError("hilo conv2 removed")

        ctx12.close()   # free conv2 weight pool

        # ---------------- phase 3: conv3
        h3p = ctx.enter_context(tc.tile_pool(name="h3p", bufs=1))
        h3 = [h3p.tile([128, 18, 18], F32, tag=f"h3_{k}", name=f"h3_{k}")
              for k in range(2)]
        for k in range(2):
            nc.vector.memset(h3[k][:], 0.0)

        b3_s = pers.tile([128, 2], F32, tag="b3")
        nc.gpsimd.dma_start(b3_s[:], b3[:])

        # prefetch ALL phase-4/5 weights during conv3 (one merged DMA)
        p4a = ctx.enter_context(tc.tile_pool(name="p4a", bufs=1))
        wtail_s = p4a.tile([128, 22016], F32, tag="wtail")
        nc.scalar.dma_start(wtail_s[:, 0:4608], wtail[:, 0:4608])
        nc.scalar.dma_start(wtail_s[:, 4608:12800], wtail[:, 4608:12800])
        nc.sync.dma_start(wtail_s[:, 12800:22016], wtail[:, 12800:22016])
        w4_s = wtail_s[:, 0:4608].rearrange("p (a b c) -> p a b c", a=2, b=9)
        rw1_s = [wtail_s[:, 4608:10752].rearrange("p (a b c) -> p a b c", a=2, b=3),
                 wtail_s[:, 12800:18944].rearrange("p (a b c) -> p a b c", a=2, b=3)]
        rw2_s = [wtail_s[:, 10752:12800].rearrange("p (a b) -> p a b", a=8),
                 wtail_s[:, 18944:20992].rearrange("p (a b) -> p a b", a=8)]
        embT_s = wtail_s[:, 20992:22016].rearrange("p (a b) -> p a b", a=2)

        TG = [(0, 13), (13, 13), (26, 13), (39, 10)]
        with tc.tile_pool(name="w3ring", bufs=3) as w3ring, \
             tc.tile_pool(name="w3tmp", bufs=2) as w3tmp, \
             tc.tile_pool(name="ps3", bufs=2, space="PSUM") as ps3:
            assert PREC3 == 'f32r', "only f32r conv3 implemented in this version"
            psh = [ps3.tile([128, 256], F32, tag=f"c3_{h}", name=f"c3_{h}")
                   for h in range(2)]
            first = True
            for kc in range(2):
                for (t0, tn) in TG:
                    w3f_t = w3tmp.tile([128, 13, 256], F32, tag="w3f", name="w3f")
                    nc.gpsimd.dma_start(w3f_t[:, :tn, :], w3f[:, kc, t0:t0 + tn])
                    w3r_t = w3ring.tile([128, 13, 256], F32R, tag="w3r", name="w3r")
                    nc.vector.tensor_copy(w3r_t[:, :tn, :], w3f_t[:, :tn, :])
                    for ti in range(tn):
                        t = t0 + ti
                        dy, dx = divmod(t, 7)
                        last = (kc == 1 and t == 48)
                        rhs = h2hi[kc][:, dy:dy + 61:4, dx:dx + 61:4]
                        for h in range(2):
                            nc.tensor.matmul(
                                psh[h][:], w3r_t[:, ti, 128 * h:128 * h + 128],
                                rhs, start=first, stop=last)
                        first = False
            for h in range(2):
                psv = psh[h][:].rearrange("p (a b) -> p a b", a=16)
                nc.scalar.activation(h3[h][:, 1:17, 1:17], psv, RELU,
                                     bias=b3_s[:, h:h + 1])

        # ---------------- phase 4: conv4 + residual blocks (fp32)
        p4 = ctx.enter_context(tc.tile_pool(name="p4", bufs=1))
        ps4 = ctx.enter_context(tc.tile_pool(name="ps4", bufs=2, space="PSUM"))

        b4_s = pers.tile([128, 2], F32, tag="b4")
        nc.gpsimd.dma_start(b4_s[:], b4[:])

        h4 = [p4.tile([128, 8, 10], F32, tag=f"h4_{k}", name=f"h4_{k}")
              for k in range(2)]
        hrel = [p4.tile([128, 8, 10], F32, tag=f"hrel_{k}", name=f"hrel_{k}")
                for k in range(2)]
        for k in range(2):
            nc.vector.memset(h4[k][:], 0.0)
            nc.vector.memset(hrel[k][:], 0.0)

        for h in range(2):
            ps = ps4.tile([128, 64], F32, tag="c4")
            first = True
            for kc in range(2):
                for t in range(9):
                    dy, dx = divmod(t, 3)
                    rhs = h3[kc][:, dy:dy + 15:2, dx:dx + 15:2]
                    nc.tensor.matmul(ps[:], w4_s[:, kc, t, 128 * h:128 * h + 128],
                                     rhs, start=first,
                                     stop=(kc == 1 and t == 8))
                    first = False
            psv = ps[:].rearrange("p (a b) -> p a b", a=8)
            nc.scalar.activation(h4[h][:, :, 1:9], psv, RELU, bias=b4_s[:, h:h + 1])

        t1r = p4.tile([128, 8, 64], F32, tag="t1r")
        for b in range(2):
            src = h4 if b == 0 else hrel
            # t1 = relu(conv1x3(src))  [1024 ch in 8 chunks]
            for m in range(8):
                ps = ps4.tile([128, 64], F32, tag="c4")
                first = True
                for kc in range(2):
                    for dx in range(3):
                        rhs = src[kc][:, :, dx:dx + 8]
                        nc.tensor.matmul(
                            ps[:], rw1_s[b][:, kc, dx, 128 * m:128 * m + 128],
                            rhs, start=first, stop=(kc == 1 and dx == 2))
                        first = False
                nc.scalar.activation(t1r[:, m, :], ps[:], RELU)
            # t2 = conv1x1(t1); h4 += t2
            for h in range(2):
                ps = ps4.tile([128, 64], F32, tag="c4")
                for kc in range(8):
                    nc.tensor.matmul(ps[:], rw2_s[b][:, kc, 128 * h:128 * h + 128],
                                     t1r[:, kc, :], start=(kc == 0), stop=(kc == 7))
                psv = ps[:].rearrange("p (a b) -> p a b", a=8)
                h4s = h4[h][:, :, 1:9]
                nc.vector.tensor_add(h4s, h4s, psv)
                if b == 0:
                    nc.scalar.activation(hrel[h][:, :, 1:9], h4s, RELU)

        # ---------------- phase 5: lat + VQ cross term
        lat = [p4.tile([128, 64], F32, tag=f"lat{k}", name=f"lat{k}")
               for k in range(2)]
        for h in range(2):
            nc.scalar.activation(
                lat[h][:].rearrange("p (a b) -> p a b", a=8),
                h4[h][:, :, 1:9], RELU)
            nc.sync.dma_start(out_lat[h], lat[h][:])

        psC = ps4.tile([64, 512], F32, tag="psC")
        for kc in range(2):
            nc.tensor.matmul(psC[:], lat[kc][:], embT_s[:, kc, :],
                             start=(kc == 0), stop=(kc == 1))
        twoC = p4.tile([64, 512], F32, tag="twoC")
        nc.scalar.activation(twoC[:], psC[:], COPY, scale=2.0)
        nc.sync.dma_start(out_twoC[:], twoC[:])

        if debug:
            for k in range(2):
                nc.sync.dma_start(di['dbg_h2hi'][k], h2hi[k][:])
                if h2lo is not None:
                    nc.sync.dma_start(di['dbg_h2lo'][k], h2lo[k][:])
                nc.sync.dma_start(di['dbg_h3'][k], h3[k][:])
                nc.sync.dma_start(di['dbg_h4'][k], h4[k][:])

    nc.compile()
    _CACHE[key] = (nc, di)
    return _CACHE[key]


# ---------------------------------------------------------------- kernel()

def kernel(x, w_in, b_in, w_h1, b_h1, w_h2, b_h2, w_h3, b_h3,
           r0_w1, r0_w2, r1_w1, r1_w2, emb, debug=False, _res_out=None):
    x = np.asarray(x, np.float32)
    emb = np.asarray(emb, np.float32)
    shared = _prep_shared(np.asarray(w_in, np.float32), np.asarray(b_in, np.float32),
                          np.asarray(w_h1, np.float32), np.asarray(b_h1, np.float32),
                          np.asarray(w_h2, np.float32), np.asarray(b_h2, np.float32),
                          np.asarray(w_h3, np.float32), np.asarray(b_h3, np.float32),
                          np.asarray(r0_w1, np.float32), np.asarray(r0_w2, np.float32),
                          np.asarray(r1_w1, np.float32), np.asarray(r1_w2, np.float32),
                          emb)
    nc, di = _build(debug=debug)

    in_maps = []
    for b in range(8):
        m = dict(shared)
        m['xim'] = _prep_x(x[b])
        in_maps.append(m)

    res = run_bass_kernel_spmd(nc, in_maps, core_ids=list(range(8)))
    if _res_out is not None:
        _res_out.append(res)

    # host: replicate reference's f32 distance + argmin, gather, losses
    lats, idxs = [], []
    B = (emb * emb).sum(axis=1)                       # f32 [512]
    for b in range(8):
        r = res.results[b]
        lat = r['out_lat'].transpose(2, 0, 1).reshape(64, 256)  # [px, D]
        A = (lat * lat).sum(axis=1, keepdims=True)    # f32 [64,1]
        twoC = r['out_twoC']                          # [64,512] f32
        dist = (A + B) - twoC                         # f32, same assoc as reference
        idx = dist.argmin(axis=1)
        lats.append(lat)
        idxs.append(idx)

    lat_all = np.concatenate(lats, axis=0)            # [512, 256]
    idx_all = np.concatenate(idxs, axis=0)            # [512]
    quant = emb[idx_all]                              # [512, 256]

    quant_st = quant.reshape(8, 8, 8, 256).transpose(0, 3, 1, 2).copy()
    diff = quant.astype(np.float64) - lat_all.astype(np.float64)
    e_loss = np.float32((diff * diff).mean())
    loss = np.float32(e_loss + BETA * e_loss)
    counts = np.bincount(idx_all, minlength=512)
    probs = (counts / 512.0).astype(np.float32)
    plog = probs * np.log(probs + np.float32(EPS), dtype=np.float32)
    perplexity = np.float32(np.exp(-plog.sum(dtype=np.float32)))

    return quant_st, loss, perplexity


# revision 15
# speedup vs baseline: 1.0092x; 1.0092x over previous
"""VQ-VAE Encoder_conv kernel for 8x Trainium2 NeuronCores (Bass/Tile).

Sharding: data-parallel over batch (8 images -> 8 cores). Each core runs the
full conv stack for one image plus the VQ distance cross-term; the host does
the final argmin (exact f32 replica of the reference arithmetic) and the
trivial emb[idx] gather.

Precision: conv1 uses bf16 hi/lo split matmuls (~2^-17 effective); conv2 and
conv3 default to float32r (~2^-13); conv4/residuals/VQ run in fp32. This
reproduces the reference's VQ argmin indices exactly (including the quantized
tie-break structure of the reference's f32 distance formula).
"""
import sys
sys.path.insert(0, '/opt/trn_rl_repo')

import numpy as np
import ml_dtypes
from contextlib import ExitStack

import concourse.bass as bass
import concourse.mybir as mybir
import concourse.tile as tile
from concourse import bacc
from concourse.bass_utils import run_bass_kernel_spmd

F32 = mybir.dt.float32
F32R = mybir.dt.float32r
BF16 = mybir.dt.bfloat16
BF = ml_dtypes.bfloat16

EPS = 1e-10
BETA = 1.0

# per-layer precision: conv2 in {'f32r','hilo'}; conv3 in {'f32r','fp32','hilo'}
PREC2 = 'f32r'
PREC3 = 'f32r'

# ---------------------------------------------------------------- host prep

def _hilo(a):
    hi = a.astype(BF)
    lo = (a.astype(np.float32) - hi.astype(np.float32)).astype(BF)
    return hi, lo


def _prep_shared(w_in, b_in, w_h1, b_h1, w_h2, b_h2, w_h3, b_h3,
                 r0_w1, r0_w2, r1_w1, r1_w2, emb):
    """Weight layout transforms (shared across all cores)."""
    d = {}
    # conv1: lhsT rows p = dx*12 + dy*3 + i  (must match x im2col partition order)
    w1 = w_in.transpose(3, 2, 1, 0).reshape(48, 128)      # (dx,dy,i) x O
    w1hi, w1lo = _hilo(w1)
    d['w1a'] = np.concatenate([w1hi, w1hi], axis=0)        # [96,128] hi block twice
    d['w1b'] = np.ascontiguousarray(w1lo)                  # [48,128]
    d['b1'] = b_in.reshape(128, 1).astype(np.float32)

    # conv2: [I=128, tap=49, O=256]
    w2 = np.ascontiguousarray(
        w_h1.transpose(2, 3, 1, 0).reshape(49, 128, 256).transpose(1, 0, 2))
    if PREC2 == 'f32r':
        d['w2f'] = w2.astype(np.float32)
    else:
        d['w2hi'], d['w2lo'] = _hilo(w2)
    d['b2'] = b_h1.reshape(2, 128).T.astype(np.float32).copy()   # [128,2]

    # conv3: [128, kc=2, tap=49, O=256]
    w3 = np.ascontiguousarray(
        w_h2.transpose(2, 3, 1, 0).reshape(49, 2, 128, 256).transpose(2, 1, 0, 3))
    if PREC3 in ('f32r', 'fp32'):
        d['w3f'] = w3.astype(np.float32)
    else:
        d['w3hi'], d['w3lo'] = _hilo(w3)
    d['b3'] = b_h2.reshape(2, 128).T.astype(np.float32).copy()

    # conv4: [128, kc=2, tap=9, O=256] f32
    w4 = np.ascontiguousarray(
        w_h3.transpose(2, 3, 1, 0).reshape(9, 2, 128, 256).transpose(2, 1, 0, 3)
    ).astype(np.float32)
    d['b4'] = b_h3.reshape(2, 128).T.astype(np.float32).copy()

    # residual blocks: rw1 [128, kc=2, dx=3, O=1024]; rw2 [128, kc=8, O=256]
    parts = [w4.reshape(128, -1)]
    for bi, (wa, wb) in enumerate(((r0_w1, r0_w2), (r1_w1, r1_w2))):
        rw1 = wa.transpose(2, 3, 1, 0).reshape(3, 2, 128, 1024).transpose(2, 1, 0, 3)
        parts.append(np.ascontiguousarray(rw1).astype(np.float32).reshape(128, -1))
        rw2 = wb[:, :, 0, 0].T.reshape(8, 128, 256).transpose(1, 0, 2)
        parts.append(np.ascontiguousarray(rw2).astype(np.float32).reshape(128, -1))

    # embT: [128, kc=2, 512]
    embt = np.ascontiguousarray(
        emb.T.reshape(2, 128, 512).transpose(1, 0, 2)
    ).astype(np.float32)
    parts.append(embt.reshape(128, -1))
    # wtail: w4(4608) rw1_0(6144) rw2_0(2048) rw1_1(6144) rw2_1(2048) embT(1024)
    d['wtail'] = np.concatenate(parts, axis=1)
    assert d['wtail'].shape == (128, 22016)
    return d


def _prep_x(xb):
    """Per-core im2col of one [3,512,512] image for conv1 (k4 s2 p1).

    Returns [96, 256, 256] bf16: partition p = dx*12 + dy*3 + i holds
    x_pad[i, 2y+dy, 2c+dx]; rows 0-47 = hi part, 48-95 = lo part.
    """
    xp = np.zeros((3, 514, 514), np.float32)
    xp[:, 1:513, 1:513] = xb
    out = np.empty((96, 256, 256), BF)
    for dx in range(4):
        for dy in range(4):
            blk = xp[:, dy:dy + 512:2, dx:dx + 512:2]      # [3,256,256]
            hi, lo = _hilo(blk)
            p = dx * 12 + dy * 3
            out[p:p + 3] = hi
            out[48 + p:48 + p + 3] = lo
    return out


# ------------------------------------------------------------- bass program

_CACHE = {}


def _build(debug=False):
    key = ('prog', debug, PREC2, PREC3)
    if key in _CACHE:
        return _CACHE[key]

    nc = bacc.Bacc("TRN2", target_bir_lowering=False, debug=False, num_devices=8)

    di = {}

    def inp(name, shape, dt):
        di[name] = nc.dram_tensor(name, list(shape), dt, kind="ExternalInput").ap()
        return di[name]

    def outp(name, shape, dt):
        di[name] = nc.dram_tensor(name, list(shape), dt, kind="ExternalOutput").ap()
        return di[name]

    xim = inp('xim', [96, 256, 256], BF16)
    w1a = inp('w1a', [96, 128], BF16)
    w1b = inp('w1b', [48, 128], BF16)
    b1 = inp('b1', [128, 1], F32)
    if PREC2 == 'f32r':
        w2f = inp('w2f', [128, 49, 256], F32)
    else:
        w2hi = inp('w2hi', [128, 49, 256], BF16)
        w2lo = inp('w2lo', [128, 49, 256], BF16)
    b2 = inp('b2', [128, 2], F32)
    if PREC3 in ('f32r', 'fp32'):
        w3f = inp('w3f', [128, 2, 49, 256], F32)
    else:
        w3hi = inp('w3hi', [128, 2, 49, 256], BF16)
        w3lo = inp('w3lo', [128, 2, 49, 256], BF16)
    b3 = inp('b3', [128, 2], F32)
    b4 = inp('b4', [128, 2], F32)
    wtail = inp('wtail', [128, 22016], F32)

    out_lat = outp('out_lat', [2, 128, 64], F32)
    out_twoC = outp('out_twoC', [64, 512], F32)
    if debug:
        h2dt = F32R if PREC2 == 'f32r' else BF16
        outp('dbg_h2hi', [2, 128, 68, 68], h2dt)
        if PREC2 != 'f32r':
            outp('dbg_h2lo', [2, 128, 68, 68], BF16)
        outp('dbg_h3', [2, 128, 18, 18], F32)
        outp('dbg_h4', [2, 128, 8, 10], F32)

    RELU = mybir.ActivationFunctionType.Relu
    COPY = mybir.ActivationFunctionType.Copy
    SUB = mybir.AluOpType.subtract
    MAXOP = mybir.AluOpType.max

    H2DT = F32R if PREC2 == 'f32r' else BF16

    with tile.TileContext(nc) as tc, ExitStack() as ctx:
        pers = ctx.enter_context(tc.tile_pool(name="pers", bufs=1))
        h2p = ctx.enter_context(tc.tile_pool(name="h2p", bufs=1))
        ctx12 = ExitStack()
        w2pool = ctx12.enter_context(tc.tile_pool(name="w2pool", bufs=1))
        # conv2 weights (f32r: DMA f32 in tap chunks, round on DVE; the tmp pool
        # lives alongside the conv1 pools so rounding overlaps conv1 compute)
        w1a_s = pers.tile([96, 128], BF16, tag="w1a")
        nc.gpsimd.dma_start(w1a_s[:], w1a[:])
        w1b_s = pers.tile([48, 128], BF16, tag="w1b")
        nc.gpsimd.dma_start(w1b_s[:], w1b[:])
        b1_s = pers.tile([128, 1], F32, tag="b1")
        nc.gpsimd.dma_start(b1_s[:], b1[:])
        b2_s = pers.tile([128, 2], F32, tag="b2")
        nc.gpsimd.dma_start(b2_s[:], b2[:])

        w2tmp = ctx12.enter_context(tc.tile_pool(name="w2tmp", bufs=1))
        w2r_s = w2pool.tile([128, 49, 256], F32R, tag="w2r")
        for c0 in range(0, 49, 13):
            cn = min(13, 49 - c0)
            w2f_t = w2tmp.tile([128, 13, 256], F32, tag="w2f", name="w2f")
            nc.gpsimd.dma_start(w2f_t[:, :cn, :], w2f[:, c0:c0 + cn])
            nc.vector.tensor_copy(w2r_s[:, c0:c0 + cn], w2f_t[:, :cn, :])

        # h2 (output of conv2, input of conv3), padded 68x68
        h2hi = [h2p.tile([128, 68, 68], H2DT, tag=f"h2hi{k}", name=f"h2hi{k}")
                for k in range(2)]
        h2lo = None
        if PREC2 != 'f32r':
            h2lo = [h2p.tile([128, 68, 68], BF16, tag=f"h2lo{k}", name=f"h2lo{k}")
                    for k in range(2)]
        for k in range(2):
            nc.vector.memset(h2hi[k][:].bitcast(F32) if PREC2 == 'f32r'
                             else h2hi[k][:], 0.0)
            if h2lo is not None:
                nc.vector.memset(h2lo[k][:], 0.0)

        # ---------------- phase 1+2: conv1 (fused strips) + conv2
        with tc.tile_pool(name="xt", bufs=1) as xtp, \
             tc.tile_pool(name="h1", bufs=2) as h1p, \
             tc.tile_pool(name="ps1", bufs=3, space="PSUM") as ps1, \
             tc.tile_pool(name="ps2", bufs=3, space="PSUM") as ps2:

            for s in range(8):
                base = 32 * s - 2                      # h1 row of tile row 0
                y_lo = max(0, base)
                y_hi = min(256, base + 35)
                R = y_hi - y_lo
                t_lo = y_lo - base                     # first valid tile row

                xt = xtp.tile([96, 35, 256], BF16, tag="xt")
                if s == 0:
                    # row-split so conv1's first chunks start after ~8 rows land
                    nc.sync.dma_start(xt[0:48, :8, :], xim[0:48, y_lo:y_lo + 8, :])
                    nc.scalar.dma_start(xt[48:96, :8, :], xim[48:96, y_lo:y_lo + 8, :])
                    nc.sync.dma_start(xt[0:48, 8:R, :], xim[0:48, y_lo + 8:y_hi, :])
                    nc.scalar.dma_start(xt[48:96, 8:R, :], xim[48:96, y_lo + 8:y_hi, :])
                else:
                    nc.sync.dma_start(xt[0:48, :R, :], xim[0:48, y_lo:y_hi, :])
                    nc.scalar.dma_start(xt[48:96, :R, :], xim[48:96, y_lo:y_hi, :])

                if PREC2 == 'f32r':
                    h1t = [h1p.tile([128, 35, 260], F32R, tag="h1r", name="h1r")]
                else:
                    h1t = [h1p.tile([128, 35, 260], BF16, tag="h1hi", name="h1hi"),
                           h1p.tile([128, 35, 260], BF16, tag="h1lo", name="h1lo")]
                for t in h1t:
                    tv = t.bitcast(F32) if PREC2 == 'f32r' else t
                    nc.vector.memset(tv[:, :, 0:2], 0.0)
                    nc.vector.memset(tv[:, :, 258:260], 0.0)
                    if t_lo > 0:
                        nc.vector.memset(tv[:, 0:t_lo, :], 0.0)
                    if t_lo + R < 35:
                        nc.vector.memset(tv[:, t_lo + R:35, :], 0.0)

                # conv1 matmuls: chunks of 2 h1 rows (N=512)
                r = 0
                while r < R:
                    nrow = min(2, R - r)
                    N = nrow * 256
                    ps = ps1.tile([128, 512], F32, tag="c1")
                    rhs = xt[:, r:r + nrow, :]
                    nc.tensor.matmul(ps[:, :N], w1a_s[:], rhs, start=True, stop=False)
                    nc.tensor.matmul(ps[:, :N], w1b_s[:], xt[0:48, r:r + nrow, :],
                                     start=False, stop=True)
                    psv = ps[:, :N].rearrange("p (a b) -> p a b", a=nrow)
                    if PREC2 == 'f32r':
                        dst = h1t[0][:, t_lo + r:t_lo + r + nrow, 2:258]
                        nc.scalar.activation(dst, psv, RELU, bias=b1_s[:])
                    else:
                        raise NotImplementedError("hilo conv2 removed")
                    r += nrow

                # conv2 for this strip: h2 rows [8s, 8s+8)
                for h in range(2):
                    ps = ps2.tile([128, 512], F32, tag="c2")
                    first = True
                    for t in range(49):
                        dy, dx = divmod(t, 7)
                        if PREC2 == 'f32r':
                            rhs = h1t[0][:, dy:dy + 29:4, dx:dx + 253:4]
                            nc.tensor.matmul(ps[:], w2r_s[:, t, 128 * h:128 * h + 128],
                                             rhs, start=first, stop=(t == 48))
                            first = False
                        else:
                            rhs_hi = h1t[0][:, dy:dy + 29:4, dx:dx + 253:4]
                            rhs_lo = h1t[1][:, dy:dy + 29:4, dx:dx + 253:4]
                            lt_hi = w2hi_s[:, t, 128 * h:128 * h + 128]
                            lt_lo = w2lo_s[:, t, 128 * h:128 * h + 128]
                            nc.tensor.matmul(ps[:], lt_hi, rhs_hi, start=first,
                                             stop=False)
                            first = False
                            nc.tensor.matmul(ps[:], lt_hi, rhs_lo, start=False,
                                             stop=False)
                            nc.tensor.matmul(ps[:], lt_lo, rhs_hi, start=False,
                                             stop=(t == 48))
                    psv = ps[:].rearrange("p (a b) -> p a b", a=8)
                    dhi = h2hi[h][:, 2 + 8 * s:10 + 8 * s, 2:66]
                    if PREC2 == 'f32r':
                        nc.scalar.activation(dhi, psv, RELU, bias=b2_s[:, h:h + 1])
                    else:
                        raise NotImplementedError("hilo conv2 removed")

        ctx12.close()   # free conv2 weight pool

        # ---------------- phase 3: conv3
        h3p = ctx.enter_context(tc.tile_pool(name="h3p", bufs=1))
        h3 = [h3p.tile([128, 18, 18], F32, tag=f"h3_{k}", name=f"h3_{k}")
              for k in range(2)]
        for k in range(2):
            nc.vector.memset(h3[k][:], 0.0)

        b3_s = pers.tile([128, 2], F32, tag="b3")
        nc.gpsimd.dma_start(b3_s[:], b3[:])

        # prefetch ALL phase-4/5 weights during conv3 (one merged DMA)
        p4a = ctx.enter_context(tc.tile_pool(name="p4a", bufs=1))
        wtail_s = p4a.tile([128, 22016], F32, tag="wtail")
        nc.scalar.dma_start(wtail_s[:, 0:4608], wtail[:, 0:4608])
        nc.scalar.dma_start(wtail_s[:, 4608:12800], wtail[:, 4608:12800])
        nc.sync.dma_start(wtail_s[:, 12800:22016], wtail[:, 12800:22016])
        w4_s = wtail_s[:, 0:4608].rearrange("p (a b c) -> p a b c", a=2, b=9)
        rw1_s = [wtail_s[:, 4608:10752].rearrange("p (a b c) -> p a b c", a=2, b=3),
                 wtail_s[:, 12800:18944].rearrange("p (a b c) -> p a b c", a=2, b=3)]
        rw2_s = [wtail_s[:, 10752:12800].rearrange("p (a b) -> p a b", a=8),
                 wtail_s[:, 18944:20992].rearrange("p (a b) -> p a b", a=8)]
        embT_s = wtail_s[:, 20992:22016].rearrange("p (a b) -> p a b", a=2)

        TG = [(0, 13), (13, 13), (26, 13), (39, 10)]
        with tc.tile_pool(name="w3ring", bufs=3) as w3ring, \
             tc.tile_pool(name="w3tmp", bufs=2) as w3tmp, \
             tc.tile_pool(name="ps3", bufs=2, space="PSUM") as ps3:
            assert PREC3 == 'f32r', "only f32r conv3 implemented in this version"
            psh = [ps3.tile([128, 256], F32, tag=f"c3_{h}", name=f"c3_{h}")
                   for h in range(2)]
            first = True
            for kc in range(2):
                for (t0, tn) in TG:
                    w3f_t = w3tmp.tile([128, 13, 256], F32, tag="w3f", name="w3f")
                    nc.gpsimd.dma_start(w3f_t[:, :tn, :], w3f[:, kc, t0:t0 + tn])
                    w3r_t = w3ring.tile([128, 13, 256], F32R, tag="w3r", name="w3r")
                    nc.vector.tensor_copy(w3r_t[:, :tn, :], w3f_t[:, :tn, :])
                    for ti in range(tn):
                        t = t0 + ti
                        dy, dx = divmod(t, 7)
                        last = (kc == 1 and t == 48)
                        rhs = h2hi[kc][:, dy:dy + 61:4, dx:dx + 61:4]
                        for h in range(2):
                            nc.tensor.matmul(
                                psh[h][:], w3r_t[:, ti, 128 * h:128 * h + 128],
                                rhs, start=first, stop=last)
                        first = False
            for h in range(2):
                psv = psh[h][:].rearrange("p (a b) -> p a b", a=16)
                nc.scalar.activation(h3[h][:, 1:17, 1:17], psv, RELU,
                                     bias=b3_s[:, h:h + 1])

        # ---------------- phase 4: conv4 + residual blocks (fp32)
        p4 = ctx.enter_context(tc.tile_pool(name="p4", bufs=1))
        ps4 = ctx.enter_context(tc.tile_pool(name="ps4", bufs=2, space="PSUM"))

        b4_s = pers.tile([128, 2], F32, tag="b4")
        nc.gpsimd.dma_start(b4_s[:], b4[:])

        h4 = [p4.tile([128, 8, 10], F32, tag=f"h4_{k}", name=f"h4_{k}")
              for k in range(2)]
        hrel = [p4.tile([128, 8, 10], F32, tag=f"hrel_{k}", name=f"hrel_{k}")
                for k in range(2)]
        for k in range(2):
            nc.vector.memset(h4[k][:], 0.0)
            nc.vector.memset(hrel[k][:], 0.0)

        for h in range(2):
            ps = ps4.tile([128, 64], F32, tag="c4")
            first = True
            for kc in range(2):
                for t in range(9):
                    dy, dx = divmod(t, 3)
                    rhs = h3[kc][:, dy:dy + 15:2, dx:dx + 15:2]
                    nc.tensor.matmul(ps[:], w4_s[:, kc, t, 128 * h:128 * h + 128],
                                     rhs, start=first,
                                     stop=(kc == 1 and t == 8))
                    first = False
            psv = ps[:].rearrange("p (a b) -> p a b", a=8)
            nc.scalar.activation(h4[h][:, :, 1:9], psv, RELU, bias=b4_s[:, h:h + 1])

        t1r = p4.tile([128, 8, 64], F32, tag="t1r")
        for b in range(2):
            src = h4 if b == 0 else hrel
            # t1 = relu(conv1x3(src))  [1024 ch in 8 chunks]
            for m in range(8):
                ps = ps4.tile([128, 64], F32, tag="c4")
                first = True
                for kc in range(2):
                    for dx in range(3):
                        rhs = src[kc][:, :, dx:dx + 8]
                        nc.tensor.matmul(
                            ps[:], rw1_s[b][:, kc, dx, 128 * m:128 * m + 128],
                            rhs, start=first, stop=(kc == 1 and dx == 2))
                        first = False
                nc.scalar.activation(t1r[:, m, :], ps[:], RELU)
            # t2 = conv1x1(t1); h4 += t2
            for h in range(2):
                ps = ps4.tile([128, 64], F32, tag="c4")
                for kc in range(8):
                    nc.tensor.matmul(ps[:], rw2_s[b][:, kc, 128 * h:128 * h + 128],
                                     t1r[:, kc, :], start=(kc == 0), stop=(kc == 7))
                psv = ps[:].rearrange("p (a b) -> p a b", a=8)
                h4s = h4[h][:, :, 1:9]
                nc.vector.tensor_add(h4s, h4s, psv)
                if b == 0:
                    nc.scalar.activation(hrel[h][:, :, 1:9], h4s, RELU)

        # ---------------- phase 5: lat + VQ cross term
        lat = [p4.tile([128, 64], F32, tag=f"lat{k}", name=f"lat{k}")
               for k in range(2)]
        for h in range(2):
            nc.scalar.activation(
                lat[h][:].rearrange("p (a b) -> p a b", a=8),
                h4[h][:, :, 1:9], RELU)
            nc.sync.dma_start(out_lat[h], lat[h][:])

        psC = ps4.tile([64, 512], F32, tag="psC")
        for kc in range(2):
            nc.tensor.matmul(psC[:], lat[kc][:], embT_s[:, kc, :],
                             start=(kc == 0), stop=(kc == 1))
        twoC = p4.tile([64, 512], F32, tag="twoC")
        nc.scalar.activation(twoC[:], psC[:], COPY, scale=2.0)
        nc.sync.dma_start(out_twoC[:], twoC[:])

        if debug:
            for k in range(2):
                nc.sync.dma_start(di['dbg_h2hi'][k], h2hi[k][:])
                if h2lo is not None:
                    nc.sync.dma_start(di['dbg_h2lo'][k], h2lo[k][:])
                nc.sync.dma_start(di['dbg_h3'][k], h3[k][:])
                nc.sync.dma_start(di['dbg_h4'][k], h4[k][:])

    nc.compile()
    _CACHE[key] = (nc, di)
    return _CACHE[key]


# ---------------------------------------------------------------- kernel()

def kernel(x, w_in, b_in, w_h1, b_h1, w_h2, b_h2, w_h3, b_h3,
           r0_w1, r0_w2, r1_w1, r1_w2, emb, debug=False, _res_out=None):
    x = np.asarray(x, np.float32)
    emb = np.asarray(emb, np.float32)
    shared = _prep_shared(np.asarray(w_in, np.float32), np.asarray(b_in, np.float32),
                          np.asarray(w_h1, np.float32), np.asarray(b_h1, np.float32),
                          np.asarray(w_h2, np.float32), np.asarray(b_h2, np.float32),
                          np.asarray(w_h3, np.float32), np.asarray(b_h3, np.float32),
                          np.asarray(r0_w1, np.float32), np.asarray(r0_w2, np.float32),
                          np.asarray(r1_w1, np.float32), np.asarray(r1_w2, np.float32),
                          emb)
    nc, di = _build(debug=debug)

    in_maps = []
    for b in range(8):
        m = dict(shared)
        m['xim'] = _prep_x(x[b])
        in_maps.append(m)

    res = run_bass_kernel_spmd(nc, in_maps, core_ids=list(range(8)))
    if _res_out is not None:
        _res_out.append(res)

    # host: replicate reference's f32 distance + argmin, gather, losses
    lats, idxs = [], []
    B = (emb * emb).sum(axis=1)                       # f32 [512]
    for b in range(8):
        r = res.results[b]
        lat = r['out_lat'].transpose(2, 0, 1).reshape(64, 256)  # [px, D]
        A = (lat * lat).sum(axis=1, keepdims=True)    # f32 [64,1]
        twoC = r['out_twoC']                          # [64,512] f32
        dist = (A + B) - twoC                         # f32, same assoc as reference
        idx = dist.argmin(axis=1)
        lats.append(lat)
        idxs.append(idx)

    lat_all = np.concatenate(lats, axis=0)            # [512, 256]
    idx_all = np.concatenate(idxs, axis=0)            # [512]
    quant = emb[idx_all]                              # [512, 256]

    quant_st = quant.reshape(8, 8, 8, 256).transpose(0, 3, 1, 2).copy()
    diff = quant.astype(np.float64) - lat_all.astype(np.float64)
    e_loss = np.float32((diff * diff).mean())
    loss = np.float32(e_loss + BETA * e_loss)
    counts = np.bincount(idx_all, minlength=512)
    probs = (counts / 512.0).astype(np.float32)
    plog = probs * np.log(probs + np.float32(EPS), dtype=np.float32)
    perplexity = np.float32(np.exp(-plog.sum(dtype=np.float32)))

    return quant_st, loss, perplexity


# revision 16
# speedup vs baseline: 1.0104x; 1.0012x over previous
"""VQ-VAE Encoder_conv kernel for 8x Trainium2 NeuronCores (Bass/Tile).

Sharding: data-parallel over batch (8 images -> 8 cores). Each core runs the
full conv stack for one image plus the VQ distance cross-term; the host does
the final argmin (exact f32 replica of the reference arithmetic) and the
trivial emb[idx] gather.

Precision: conv1 uses bf16 hi/lo split matmuls (~2^-17 effective); conv2 and
conv3 default to float32r (~2^-13); conv4/residuals/VQ run in fp32. This
reproduces the reference's VQ argmin indices exactly (including the quantized
tie-break structure of the reference's f32 distance formula).
"""
import sys
sys.path.insert(0, '/opt/trn_rl_repo')

import numpy as np
import ml_dtypes
from contextlib import ExitStack

import concourse.bass as bass
import concourse.mybir as mybir
import concourse.tile as tile
from concourse import bacc
from concourse.bass_utils import run_bass_kernel_spmd

F32 = mybir.dt.float32
F32R = mybir.dt.float32r
BF16 = mybir.dt.bfloat16
BF = ml_dtypes.bfloat16

EPS = 1e-10
BETA = 1.0

# per-layer precision: conv2 in {'f32r','hilo'}; conv3 in {'f32r','fp32','hilo'}
PREC2 = 'f32r'
PREC3 = 'f32r'

# ---------------------------------------------------------------- host prep

def _hilo(a):
    hi = a.astype(BF)
    lo = (a.astype(np.float32) - hi.astype(np.float32)).astype(BF)
    return hi, lo


def _prep_shared(w_in, b_in, w_h1, b_h1, w_h2, b_h2, w_h3, b_h3,
                 r0_w1, r0_w2, r1_w1, r1_w2, emb):
    """Weight layout transforms (shared across all cores)."""
    d = {}
    # conv1: lhsT rows p = dx*12 + dy*3 + i  (must match x im2col partition order)
    w1 = w_in.transpose(3, 2, 1, 0).reshape(48, 128)      # (dx,dy,i) x O
    w1hi, w1lo = _hilo(w1)
    d['w1a'] = np.concatenate([w1hi, w1hi], axis=0)        # [96,128] hi block twice
    d['w1b'] = np.ascontiguousarray(w1lo)                  # [48,128]
    d['b1'] = b_in.reshape(128, 1).astype(np.float32)

    # conv2: [I=128, tap=49, O=256]
    w2 = np.ascontiguousarray(
        w_h1.transpose(2, 3, 1, 0).reshape(49, 128, 256).transpose(1, 0, 2))
    if PREC2 == 'f32r':
        d['w2f'] = w2.astype(np.float32)
    else:
        d['w2hi'], d['w2lo'] = _hilo(w2)
    d['b2'] = b_h1.reshape(2, 128).T.astype(np.float32).copy()   # [128,2]

    # conv3: [128, kc=2, tap=49, O=256]
    w3 = np.ascontiguousarray(
        w_h2.transpose(2, 3, 1, 0).reshape(49, 2, 128, 256).transpose(2, 1, 0, 3))
    if PREC3 in ('f32r', 'fp32'):
        d['w3f'] = w3.astype(np.float32)
    else:
        d['w3hi'], d['w3lo'] = _hilo(w3)
    d['b3'] = b_h2.reshape(2, 128).T.astype(np.float32).copy()

    # conv4: [128, kc=2, tap=9, O=256] f32
    w4 = np.ascontiguousarray(
        w_h3.transpose(2, 3, 1, 0).reshape(9, 2, 128, 256).transpose(2, 1, 0, 3)
    ).astype(np.float32)
    d['b4'] = b_h3.reshape(2, 128).T.astype(np.float32).copy()

    # residual blocks: rw1 [128, kc=2, dx=3, O=1024]; rw2 [128, kc=8, O=256]
    parts = [w4.reshape(128, -1)]
    for bi, (wa, wb) in enumerate(((r0_w1, r0_w2), (r1_w1, r1_w2))):
        rw1 = wa.transpose(2, 3, 1, 0).reshape(3, 2, 128, 1024).transpose(2, 1, 0, 3)
        parts.append(np.ascontiguousarray(rw1).astype(np.float32).reshape(128, -1))
        rw2 = wb[:, :, 0, 0].T.reshape(8, 128, 256).transpose(1, 0, 2)
        parts.append(np.ascontiguousarray(rw2).astype(np.float32).reshape(128, -1))

    # embT: [128, kc=2, 512]
    embt = np.ascontiguousarray(
        emb.T.reshape(2, 128, 512).transpose(1, 0, 2)
    ).astype(np.float32)
    parts.append(embt.reshape(128, -1))
    # wtail: w4(4608) rw1_0(6144) rw2_0(2048) rw1_1(6144) rw2_1(2048) embT(1024)
    d['wtail'] = np.concatenate(parts, axis=1)
    assert d['wtail'].shape == (128, 22016)
    return d


def _prep_x(xb):
    """Per-core im2col of one [3,512,512] image for conv1 (k4 s2 p1).

    Returns [96, 256, 256] bf16: partition p = dx*12 + dy*3 + i holds
    x_pad[i, 2y+dy, 2c+dx]; rows 0-47 = hi part, 48-95 = lo part.
    """
    xp = np.zeros((3, 514, 514), np.float32)
    xp[:, 1:513, 1:513] = xb
    out = np.empty((96, 256, 256), BF)
    for dx in range(4):
        for dy in range(4):
            blk = xp[:, dy:dy + 512:2, dx:dx + 512:2]      # [3,256,256]
            hi, lo = _hilo(blk)
            p = dx * 12 + dy * 3
            out[p:p + 3] = hi
            out[48 + p:48 + p + 3] = lo
    return out


# ------------------------------------------------------------- bass program

_CACHE = {}


def _build(debug=False):
    key = ('prog', debug, PREC2, PREC3)
    if key in _CACHE:
        return _CACHE[key]

    nc = bacc.Bacc("TRN2", target_bir_lowering=False, debug=False, num_devices=8)

    di = {}

    def inp(name, shape, dt):
        di[name] = nc.dram_tensor(name, list(shape), dt, kind="ExternalInput").ap()
        return di[name]

    def outp(name, shape, dt):
        di[name] = nc.dram_tensor(name, list(shape), dt, kind="ExternalOutput").ap()
        return di[name]

    xim = inp('xim', [96, 256, 256], BF16)
    w1a = inp('w1a', [96, 128], BF16)
    w1b = inp('w1b', [48, 128], BF16)
    b1 = inp('b1', [128, 1], F32)
    if PREC2 == 'f32r':
        w2f = inp('w2f', [128, 49, 256], F32)
    else:
        w2hi = inp('w2hi', [128, 49, 256], BF16)
        w2lo = inp('w2lo', [128, 49, 256], BF16)
    b2 = inp('b2', [128, 2], F32)
    if PREC3 in ('f32r', 'fp32'):
        w3f = inp('w3f', [128, 2, 49, 256], F32)
    else:
        w3hi = inp('w3hi', [128, 2, 49, 256], BF16)
        w3lo = inp('w3lo', [128, 2, 49, 256], BF16)
    b3 = inp('b3', [128, 2], F32)
    b4 = inp('b4', [128, 2], F32)
    wtail = inp('wtail', [128, 22016], F32)

    out_lat = outp('out_lat', [2, 128, 64], F32)
    out_twoC = outp('out_twoC', [64, 512], F32)
    if debug:
        h2dt = F32R if PREC2 == 'f32r' else BF16
        outp('dbg_h2hi', [2, 128, 68, 68], h2dt)
        if PREC2 != 'f32r':
            outp('dbg_h2lo', [2, 128, 68, 68], BF16)
        outp('dbg_h3', [2, 128, 18, 18], F32)
        outp('dbg_h4', [2, 128, 8, 10], F32)

    RELU = mybir.ActivationFunctionType.Relu
    COPY = mybir.ActivationFunctionType.Copy
    SUB = mybir.AluOpType.subtract
    MAXOP = mybir.AluOpType.max

    H2DT = F32R if PREC2 == 'f32r' else BF16

    with tile.TileContext(nc) as tc, ExitStack() as ctx:
        pers = ctx.enter_context(tc.tile_pool(name="pers", bufs=1))
        h2p = ctx.enter_context(tc.tile_pool(name="h2p", bufs=1))
        ctx12 = ExitStack()
        w2pool = ctx12.enter_context(tc.tile_pool(name="w2pool", bufs=1))
        # conv2 weights (f32r: DMA f32 in tap chunks, round on DVE; the tmp pool
        # lives alongside the conv1 pools so rounding overlaps conv1 compute)
        w1a_s = pers.tile([96, 128], BF16, tag="w1a")
        nc.gpsimd.dma_start(w1a_s[:], w1a[:])
        w1b_s = pers.tile([48, 128], BF16, tag="w1b")
        nc.gpsimd.dma_start(w1b_s[:], w1b[:])
        b1_s = pers.tile([128, 1], F32, tag="b1")
        nc.gpsimd.dma_start(b1_s[:], b1[:])
        b2_s = pers.tile([128, 2], F32, tag="b2")
        nc.gpsimd.dma_start(b2_s[:], b2[:])

        w2tmp = ctx12.enter_context(tc.tile_pool(name="w2tmp", bufs=1))
        w2r_s = w2pool.tile([128, 49, 256], F32R, tag="w2r")
        for c0 in range(0, 49, 13):
            cn = min(13, 49 - c0)
            w2f_t = w2tmp.tile([128, 13, 256], F32, tag="w2f", name="w2f")
            nc.gpsimd.dma_start(w2f_t[:, :cn, :], w2f[:, c0:c0 + cn])
            nc.vector.tensor_copy(w2r_s[:, c0:c0 + cn], w2f_t[:, :cn, :])

        # h2 (output of conv2, input of conv3), padded 68x68
        h2hi = [h2p.tile([128, 68, 68], H2DT, tag=f"h2hi{k}", name=f"h2hi{k}")
                for k in range(2)]
        h2lo = None
        if PREC2 != 'f32r':
            h2lo = [h2p.tile([128, 68, 68], BF16, tag=f"h2lo{k}", name=f"h2lo{k}")
                    for k in range(2)]
        for k in range(2):
            nc.vector.memset(h2hi[k][:].bitcast(F32) if PREC2 == 'f32r'
                             else h2hi[k][:], 0.0)
            if h2lo is not None:
                nc.vector.memset(h2lo[k][:], 0.0)

        # ---------------- phase 1+2: conv1 (fused strips) + conv2
        with tc.tile_pool(name="xt", bufs=1) as xtp, \
             tc.tile_pool(name="h1", bufs=2) as h1p, \
             tc.tile_pool(name="ps1", bufs=3, space="PSUM") as ps1, \
             tc.tile_pool(name="ps2", bufs=3, space="PSUM") as ps2:

            for s in range(8):
                base = 32 * s - 2                      # h1 row of tile row 0
                y_lo = max(0, base)
                y_hi = min(256, base + 35)
                R = y_hi - y_lo
                t_lo = y_lo - base                     # first valid tile row

                xt = xtp.tile([96, 35, 256], BF16, tag="xt")
                if s == 0:
                    # row-split so conv1's first chunks start after ~8 rows land
                    nc.sync.dma_start(xt[0:48, :8, :], xim[0:48, y_lo:y_lo + 8, :])
                    nc.scalar.dma_start(xt[48:96, :8, :], xim[48:96, y_lo:y_lo + 8, :])
                    nc.sync.dma_start(xt[0:48, 8:R, :], xim[0:48, y_lo + 8:y_hi, :])
                    nc.scalar.dma_start(xt[48:96, 8:R, :], xim[48:96, y_lo + 8:y_hi, :])
                else:
                    nc.sync.dma_start(xt[0:48, :R, :], xim[0:48, y_lo:y_hi, :])
                    nc.scalar.dma_start(xt[48:96, :R, :], xim[48:96, y_lo:y_hi, :])

                if PREC2 == 'f32r':
                    h1t = [h1p.tile([128, 35, 260], F32R, tag="h1r", name="h1r")]
                else:
                    h1t = [h1p.tile([128, 35, 260], BF16, tag="h1hi", name="h1hi"),
                           h1p.tile([128, 35, 260], BF16, tag="h1lo", name="h1lo")]
                for t in h1t:
                    tv = t.bitcast(F32) if PREC2 == 'f32r' else t
                    nc.vector.memset(tv[:, :, 0:2], 0.0)
                    nc.vector.memset(tv[:, :, 258:260], 0.0)
                    if t_lo > 0:
                        nc.vector.memset(tv[:, 0:t_lo, :], 0.0)
                    if t_lo + R < 35:
                        nc.vector.memset(tv[:, t_lo + R:35, :], 0.0)

                # conv1 matmuls: chunks of 2 h1 rows (N=512)
                r = 0
                while r < R:
                    nrow = min(2, R - r)
                    N = nrow * 256
                    ps = ps1.tile([128, 512], F32, tag="c1")
                    rhs = xt[:, r:r + nrow, :]
                    nc.tensor.matmul(ps[:, :N], w1a_s[:], rhs, start=True, stop=False)
                    nc.tensor.matmul(ps[:, :N], w1b_s[:], xt[0:48, r:r + nrow, :],
                                     start=False, stop=True)
                    psv = ps[:, :N].rearrange("p (a b) -> p a b", a=nrow)
                    if PREC2 == 'f32r':
                        dst = h1t[0][:, t_lo + r:t_lo + r + nrow, 2:258]
                        nc.scalar.activation(dst, psv, RELU, bias=b1_s[:])
                    else:
                        raise NotImplementedError("hilo conv2 removed")
                    r += nrow

                # conv2 for this strip: h2 rows [8s, 8s+8)
                for h in range(2):
                    ps = ps2.tile([128, 512], F32, tag="c2")
                    first = True
                    for t in range(49):
                        dy, dx = divmod(t, 7)
                        if PREC2 == 'f32r':
                            rhs = h1t[0][:, dy:dy + 29:4, dx:dx + 253:4]
                            nc.tensor.matmul(ps[:], w2r_s[:, t, 128 * h:128 * h + 128],
                                             rhs, start=first, stop=(t == 48))
                            first = False
                        else:
                            rhs_hi = h1t[0][:, dy:dy + 29:4, dx:dx + 253:4]
                            rhs_lo = h1t[1][:, dy:dy + 29:4, dx:dx + 253:4]
                            lt_hi = w2hi_s[:, t, 128 * h:128 * h + 128]
                            lt_lo = w2lo_s[:, t, 128 * h:128 * h + 128]
                            nc.tensor.matmul(ps[:], lt_hi, rhs_hi, start=first,
                                             stop=False)
                            first = False
                            nc.tensor.matmul(ps[:], lt_hi, rhs_lo, start=False,
                                             stop=False)
                            nc.tensor.matmul(ps[:], lt_lo, rhs_hi, start=False,
                                             stop=(t == 48))
                    psv = ps[:].rearrange("p (a b) -> p a b", a=8)
                    dhi = h2hi[h][:, 2 + 8 * s:10 + 8 * s, 2:66]
                    if PREC2 == 'f32r':
                        nc.scalar.activation(dhi, psv, RELU, bias=b2_s[:, h:h + 1])
                    else:
                        raise NotImplementedError("hilo conv2 removed")

        ctx12.close()   # free conv2 weight pool

        # ---------------- phase 3: conv3
        h3p = ctx.enter_context(tc.tile_pool(name="h3p", bufs=1))
        h3 = [h3p.tile([128, 18, 18], F32, tag=f"h3_{k}", name=f"h3_{k}")
              for k in range(2)]
        for k in range(2):
            nc.vector.memset(h3[k][:], 0.0)

        b3_s = pers.tile([128, 2], F32, tag="b3")
        nc.gpsimd.dma_start(b3_s[:], b3[:])

        # prefetch ALL phase-4/5 weights during conv3 (one merged DMA)
        p4a = ctx.enter_context(tc.tile_pool(name="p4a", bufs=1))
        wtail_s = p4a.tile([128, 22016], F32, tag="wtail")
        nc.scalar.dma_start(wtail_s[:, 0:4608], wtail[:, 0:4608])
        nc.scalar.dma_start(wtail_s[:, 4608:12800], wtail[:, 4608:12800])
        nc.sync.dma_start(wtail_s[:, 12800:22016], wtail[:, 12800:22016])
        w4_s = wtail_s[:, 0:4608].rearrange("p (a b c) -> p a b c", a=2, b=9)
        rw1_s = [wtail_s[:, 4608:10752].rearrange("p (a b c) -> p a b c", a=2, b=3),
                 wtail_s[:, 12800:18944].rearrange("p (a b c) -> p a b c", a=2, b=3)]
        rw2_s = [wtail_s[:, 10752:12800].rearrange("p (a b) -> p a b", a=8),
                 wtail_s[:, 18944:20992].rearrange("p (a b) -> p a b", a=8)]
        embT_s = wtail_s[:, 20992:22016].rearrange("p (a b) -> p a b", a=2)

        # phase-4 pools created before conv3's scoped pools so conv4's
        # eviction targets and PSUM banks don't wait on conv3 teardown
        p4 = ctx.enter_context(tc.tile_pool(name="p4", bufs=1))
        ps4 = ctx.enter_context(tc.tile_pool(name="ps4", bufs=2, space="PSUM"))
        b4_s = pers.tile([128, 2], F32, tag="b4")
        nc.gpsimd.dma_start(b4_s[:], b4[:])
        h4 = [p4.tile([128, 8, 10], F32, tag=f"h4_{k}", name=f"h4_{k}")
              for k in range(2)]
        hrel = [p4.tile([128, 8, 10], F32, tag=f"hrel_{k}", name=f"hrel_{k}")
                for k in range(2)]
        for k in range(2):
            nc.vector.memset(h4[k][:], 0.0)
            nc.vector.memset(hrel[k][:], 0.0)

        TG = [(0, 13), (13, 13), (26, 13), (39, 10)]
        with tc.tile_pool(name="w3ring", bufs=3) as w3ring, \
             tc.tile_pool(name="w3tmp", bufs=2) as w3tmp, \
             tc.tile_pool(name="ps3", bufs=2, space="PSUM") as ps3:
            assert PREC3 == 'f32r', "only f32r conv3 implemented in this version"
            psh = [ps3.tile([128, 256], F32, tag=f"c3_{h}", name=f"c3_{h}")
                   for h in range(2)]
            first = True
            for kc in range(2):
                for (t0, tn) in TG:
                    w3f_t = w3tmp.tile([128, 13, 256], F32, tag="w3f", name="w3f")
                    nc.gpsimd.dma_start(w3f_t[:, :tn, :], w3f[:, kc, t0:t0 + tn])
                    w3r_t = w3ring.tile([128, 13, 256], F32R, tag="w3r", name="w3r")
                    nc.vector.tensor_copy(w3r_t[:, :tn, :], w3f_t[:, :tn, :])
                    for ti in range(tn):
                        t = t0 + ti
                        dy, dx = divmod(t, 7)
                        last = (kc == 1 and t == 48)
                        rhs = h2hi[kc][:, dy:dy + 61:4, dx:dx + 61:4]
                        for h in range(2):
                            nc.tensor.matmul(
                                psh[h][:], w3r_t[:, ti, 128 * h:128 * h + 128],
                                rhs, start=first, stop=last)
                        first = False
            for h in range(2):
                psv = psh[h][:].rearrange("p (a b) -> p a b", a=16)
                nc.scalar.activation(h3[h][:, 1:17, 1:17], psv, RELU,
                                     bias=b3_s[:, h:h + 1])

        # ---------------- phase 4: conv4 + residual blocks (fp32)

        for h in range(2):
            ps = ps4.tile([128, 64], F32, tag="c4")
            first = True
            for kc in range(2):
                for t in range(9):
                    dy, dx = divmod(t, 3)
                    rhs = h3[kc][:, dy:dy + 15:2, dx:dx + 15:2]
                    nc.tensor.matmul(ps[:], w4_s[:, kc, t, 128 * h:128 * h + 128],
                                     rhs, start=first,
                                     stop=(kc == 1 and t == 8))
                    first = False
            psv = ps[:].rearrange("p (a b) -> p a b", a=8)
            nc.scalar.activation(h4[h][:, :, 1:9], psv, RELU, bias=b4_s[:, h:h + 1])

        t1r = p4.tile([128, 8, 64], F32, tag="t1r")
        for b in range(2):
            src = h4 if b == 0 else hrel
            # t1 = relu(conv1x3(src))  [1024 ch in 8 chunks]
            for m in range(8):
                ps = ps4.tile([128, 64], F32, tag="c4")
                first = True
                for kc in range(2):
                    for dx in range(3):
                        rhs = src[kc][:, :, dx:dx + 8]
                        nc.tensor.matmul(
                            ps[:], rw1_s[b][:, kc, dx, 128 * m:128 * m + 128],
                            rhs, start=first, stop=(kc == 1 and dx == 2))
                        first = False
                nc.scalar.activation(t1r[:, m, :], ps[:], RELU)
            # t2 = conv1x1(t1); h4 += t2
            for h in range(2):
                ps = ps4.tile([128, 64], F32, tag="c4")
                for kc in range(8):
                    nc.tensor.matmul(ps[:], rw2_s[b][:, kc, 128 * h:128 * h + 128],
                                     t1r[:, kc, :], start=(kc == 0), stop=(kc == 7))
                psv = ps[:].rearrange("p (a b) -> p a b", a=8)
                h4s = h4[h][:, :, 1:9]
                nc.vector.tensor_add(h4s, h4s, psv)
                if b == 0:
                    nc.scalar.activation(hrel[h][:, :, 1:9], h4s, RELU)

        # ---------------- phase 5: lat + VQ cross term
        lat = [p4.tile([128, 64], F32, tag=f"lat{k}", name=f"lat{k}")
               for k in range(2)]
        for h in range(2):
            nc.scalar.activation(
                lat[h][:].rearrange("p (a b) -> p a b", a=8),
                h4[h][:, :, 1:9], RELU)
            nc.sync.dma_start(out_lat[h], lat[h][:])

        psC = ps4.tile([64, 512], F32, tag="psC")
        for kc in range(2):
            nc.tensor.matmul(psC[:], lat[kc][:], embT_s[:, kc, :],
                             start=(kc == 0), stop=(kc == 1))
        twoC = p4.tile([64, 512], F32, tag="twoC")
        nc.scalar.activation(twoC[:], psC[:], COPY, scale=2.0)
        nc.sync.dma_start(out_twoC[:], twoC[:])

        if debug:
            for k in range(2):
                nc.sync.dma_start(di['dbg_h2hi'][k], h2hi[k][:])
                if h2lo is not None:
                    nc.sync.dma_start(di['dbg_h2lo'][k], h2lo[k][:])
                nc.sync.dma_start(di['dbg_h3'][k], h3[k][:])
                nc.sync.dma_start(di['dbg_h4'][k], h4[k][:])

    nc.compile()
    _CACHE[key] = (nc, di)
    return _CACHE[key]


# ---------------------------------------------------------------- kernel()

def kernel(x, w_in, b_in, w_h1, b_h1, w_h2, b_h2, w_h3, b_h3,
           r0_w1, r0_w2, r1_w1, r1_w2, emb, debug=False, _res_out=None):
    x = np.asarray(x, np.float32)
    emb = np.asarray(emb, np.float32)
    shared = _prep_shared(np.asarray(w_in, np.float32), np.asarray(b_in, np.float32),
                          np.asarray(w_h1, np.float32), np.asarray(b_h1, np.float32),
                          np.asarray(w_h2, np.float32), np.asarray(b_h2, np.float32),
                          np.asarray(w_h3, np.float32), np.asarray(b_h3, np.float32),
                          np.asarray(r0_w1, np.float32), np.asarray(r0_w2, np.float32),
                          np.asarray(r1_w1, np.float32), np.asarray(r1_w2, np.float32),
                          emb)
    nc, di = _build(debug=debug)

    in_maps = []
    for b in range(8):
        m = dict(shared)
        m['xim'] = _prep_x(x[b])
        in_maps.append(m)

    res = run_bass_kernel_spmd(nc, in_maps, core_ids=list(range(8)))
    if _res_out is not None:
        _res_out.append(res)

    # host: replicate reference's f32 distance + argmin, gather, losses
    lats, idxs = [], []
    B = (emb * emb).sum(axis=1)                       # f32 [512]
    for b in range(8):
        r = res.results[b]
        lat = r['out_lat'].transpose(2, 0, 1).reshape(64, 256)  # [px, D]
        A = (lat * lat).sum(axis=1, keepdims=True)    # f32 [64,1]
        twoC = r['out_twoC']                          # [64,512] f32
        dist = (A + B) - twoC                         # f32, same assoc as reference
        idx = dist.argmin(axis=1)
        lats.append(lat)
        idxs.append(idx)

    lat_all = np.concatenate(lats, axis=0)            # [512, 256]
    idx_all = np.concatenate(idxs, axis=0)            # [512]
    quant = emb[idx_all]                              # [512, 256]

    quant_st = quant.reshape(8, 8, 8, 256).transpose(0, 3, 1, 2).copy()
    diff = quant.astype(np.float64) - lat_all.astype(np.float64)
    e_loss = np.float32((diff * diff).mean())
    loss = np.float32(e_loss + BETA * e_loss)
    counts = np.bincount(idx_all, minlength=512)
    probs = (counts / 512.0).astype(np.float32)
    plog = probs * np.log(probs + np.float32(EPS), dtype=np.float32)
    perplexity = np.float32(np.exp(-plog.sum(dtype=np.float32)))

    return quant_st, loss, perplexity


# revision 17
# speedup vs baseline: 1.0331x; 1.0224x over previous
"""VQ-VAE Encoder_conv kernel for 8x Trainium2 NeuronCores (Bass/Tile).

Sharding: data-parallel over batch (8 images -> 8 cores). Each core runs the
full conv stack for one image plus the VQ distance cross-term; the host does
the final argmin (exact f32 replica of the reference arithmetic) and the
trivial emb[idx] gather.

Precision: conv1 uses bf16 hi/lo split matmuls (~2^-17 effective); conv2 and
conv3 default to float32r (~2^-13); conv4/residuals/VQ run in fp32. This
reproduces the reference's VQ argmin indices exactly (including the quantized
tie-break structure of the reference's f32 distance formula).
"""
import sys
sys.path.insert(0, '/opt/trn_rl_repo')

import numpy as np
import ml_dtypes
from contextlib import ExitStack

import concourse.bass as bass
import concourse.mybir as mybir
import concourse.tile as tile
from concourse import bacc
from concourse.bass_utils import run_bass_kernel_spmd

F32 = mybir.dt.float32
F32R = mybir.dt.float32r
BF16 = mybir.dt.bfloat16
BF = ml_dtypes.bfloat16

EPS = 1e-10
BETA = 1.0

# per-layer precision: conv2 in {'f32r','hilo'}; conv3 in {'f32r','fp32','hilo'}
PREC2 = 'f32r'
PREC3 = 'f32r'

# ---------------------------------------------------------------- host prep

def _hilo(a):
    hi = a.astype(BF)
    lo = (a.astype(np.float32) - hi.astype(np.float32)).astype(BF)
    return hi, lo


def _prep_shared(w_in, b_in, w_h1, b_h1, w_h2, b_h2, w_h3, b_h3,
                 r0_w1, r0_w2, r1_w1, r1_w2, emb):
    """Weight layout transforms (shared across all cores)."""
    d = {}
    # conv1: lhsT rows p = dx*12 + dy*3 + i  (must match x im2col partition order)
    w1 = w_in.transpose(3, 2, 1, 0).reshape(48, 128)      # (dx,dy,i) x O
    w1hi, w1lo = _hilo(w1)
    d['w1a'] = np.concatenate([w1hi, w1hi], axis=0)        # [96,128] hi block twice
    d['w1b'] = np.ascontiguousarray(w1lo)                  # [48,128]
    d['b1'] = b_in.reshape(128, 1).astype(np.float32)

    # conv2: [I=128, tap=49, O=256]
    w2 = np.ascontiguousarray(
        w_h1.transpose(2, 3, 1, 0).reshape(49, 128, 256).transpose(1, 0, 2))
    if PREC2 == 'f32r':
        d['w2f'] = w2.astype(np.float32)
    else:
        d['w2hi'], d['w2lo'] = _hilo(w2)
    d['b2'] = b_h1.reshape(2, 128).T.astype(np.float32).copy()   # [128,2]

    # conv3: [128, kc=2, tap=49, O=256]
    w3 = np.ascontiguousarray(
        w_h2.transpose(2, 3, 1, 0).reshape(49, 2, 128, 256).transpose(2, 1, 0, 3))
    if PREC3 in ('f32r', 'fp32'):
        d['w3f'] = w3.astype(np.float32)
    else:
        d['w3hi'], d['w3lo'] = _hilo(w3)
    d['b3'] = b_h2.reshape(2, 128).T.astype(np.float32).copy()

    # conv4: [128, kc=2, tap=9, O=256] f32
    w4 = np.ascontiguousarray(
        w_h3.transpose(2, 3, 1, 0).reshape(9, 2, 128, 256).transpose(2, 1, 0, 3)
    ).astype(np.float32)
    d['b4'] = b_h3.reshape(2, 128).T.astype(np.float32).copy()

    # residual blocks: rw1 [128, kc=2, dx=3, O=1024]; rw2 [128, kc=8, O=256]
    parts = [w4.reshape(128, -1)]
    for bi, (wa, wb) in enumerate(((r0_w1, r0_w2), (r1_w1, r1_w2))):
        rw1 = wa.transpose(2, 3, 1, 0).reshape(3, 2, 128, 1024).transpose(2, 1, 0, 3)
        parts.append(np.ascontiguousarray(rw1).astype(np.float32).reshape(128, -1))
        rw2 = wb[:, :, 0, 0].T.reshape(8, 128, 256).transpose(1, 0, 2)
        parts.append(np.ascontiguousarray(rw2).astype(np.float32).reshape(128, -1))

    # embT: [128, kc=2, 512]
    embt = np.ascontiguousarray(
        emb.T.reshape(2, 128, 512).transpose(1, 0, 2)
    ).astype(np.float32)
    parts.append(embt.reshape(128, -1))
    # wtail: w4(4608) rw1_0(6144) rw2_0(2048) rw1_1(6144) rw2_1(2048) embT(1024)
    d['wtail'] = np.concatenate(parts, axis=1)
    assert d['wtail'].shape == (128, 22016)
    return d


def _prep_x(xb):
    """Per-core im2col of one [3,512,512] image for conv1 (k4 s2 p1).

    Returns [96, 256, 256] bf16: partition p = dx*12 + dy*3 + i holds
    x_pad[i, 2y+dy, 2c+dx]; rows 0-47 = hi part, 48-95 = lo part.
    """
    xp = np.zeros((3, 514, 514), np.float32)
    xp[:, 1:513, 1:513] = xb
    out = np.empty((96, 256, 256), BF)
    for dx in range(4):
        for dy in range(4):
            blk = xp[:, dy:dy + 512:2, dx:dx + 512:2]      # [3,256,256]
            hi, lo = _hilo(blk)
            p = dx * 12 + dy * 3
            out[p:p + 3] = hi
            out[48 + p:48 + p + 3] = lo
    return out


# ------------------------------------------------------------- bass program

_CACHE = {}


def _build(debug=False):
    key = ('prog', debug, PREC2, PREC3)
    if key in _CACHE:
        return _CACHE[key]

    nc = bacc.Bacc("TRN2", target_bir_lowering=False, debug=False, num_devices=8)

    di = {}

    def inp(name, shape, dt):
        di[name] = nc.dram_tensor(name, list(shape), dt, kind="ExternalInput").ap()
        return di[name]

    def outp(name, shape, dt):
        di[name] = nc.dram_tensor(name, list(shape), dt, kind="ExternalOutput").ap()
        return di[name]

    xim = inp('xim', [96, 256, 256], BF16)
    w1a = inp('w1a', [96, 128], BF16)
    w1b = inp('w1b', [48, 128], BF16)
    b1 = inp('b1', [128, 1], F32)
    if PREC2 == 'f32r':
        w2f = inp('w2f', [128, 49, 256], F32)
    else:
        w2hi = inp('w2hi', [128, 49, 256], BF16)
        w2lo = inp('w2lo', [128, 49, 256], BF16)
    b2 = inp('b2', [128, 2], F32)
    if PREC3 in ('f32r', 'fp32'):
        w3f = inp('w3f', [128, 2, 49, 256], F32)
    else:
        w3hi = inp('w3hi', [128, 2, 49, 256], BF16)
        w3lo = inp('w3lo', [128, 2, 49, 256], BF16)
    b3 = inp('b3', [128, 2], F32)
    b4 = inp('b4', [128, 2], F32)
    wtail = inp('wtail', [128, 22016], F32)

    out_lat = outp('out_lat', [2, 128, 64], F32)
    out_twoC = outp('out_twoC', [64, 512], F32)
    if debug:
        h2dt = F32R if PREC2 == 'f32r' else BF16
        outp('dbg_h2hi', [2, 128, 68, 68], h2dt)
        if PREC2 != 'f32r':
            outp('dbg_h2lo', [2, 128, 68, 68], BF16)
        outp('dbg_h3', [2, 128, 18, 18], F32)
        outp('dbg_h4', [2, 128, 8, 10], F32)

    RELU = mybir.ActivationFunctionType.Relu
    COPY = mybir.ActivationFunctionType.Copy
    SUB = mybir.AluOpType.subtract
    MAXOP = mybir.AluOpType.max

    H2DT = F32R if PREC2 == 'f32r' else BF16

    with tile.TileContext(nc) as tc, ExitStack() as ctx:
        pers = ctx.enter_context(tc.tile_pool(name="pers", bufs=1))
        h2p = ctx.enter_context(tc.tile_pool(name="h2p", bufs=1))
        ctx12 = ExitStack()
        w2pool = ctx12.enter_context(tc.tile_pool(name="w2pool", bufs=1))
        # conv2 weights (f32r: DMA f32 in tap chunks, round on DVE; the tmp pool
        # lives alongside the conv1 pools so rounding overlaps conv1 compute)
        w1a_s = pers.tile([96, 128], BF16, tag="w1a")
        nc.gpsimd.dma_start(w1a_s[:], w1a[:])
        w1b_s = pers.tile([48, 128], BF16, tag="w1b")
        nc.gpsimd.dma_start(w1b_s[:], w1b[:])
        b1_s = pers.tile([128, 1], F32, tag="b1")
        nc.gpsimd.dma_start(b1_s[:], b1[:])
        b2_s = pers.tile([128, 2], F32, tag="b2")
        nc.gpsimd.dma_start(b2_s[:], b2[:])

        w2tmp = ctx12.enter_context(tc.tile_pool(name="w2tmp", bufs=1))
        w2r_s = w2pool.tile([128, 49, 256], F32R, tag="w2r")
        for c0 in range(0, 49, 13):
            cn = min(13, 49 - c0)
            w2f_t = w2tmp.tile([128, 13, 256], F32, tag="w2f", name="w2f")
            nc.gpsimd.dma_start(w2f_t[:, :cn, :], w2f[:, c0:c0 + cn])
            nc.vector.tensor_copy(w2r_s[:, c0:c0 + cn], w2f_t[:, :cn, :])

        # h2 (output of conv2, input of conv3), padded 68x68
        h2hi = [h2p.tile([128, 68, 68], H2DT, tag=f"h2hi{k}", name=f"h2hi{k}")
                for k in range(2)]
        h2lo = None
        if PREC2 != 'f32r':
            h2lo = [h2p.tile([128, 68, 68], BF16, tag=f"h2lo{k}", name=f"h2lo{k}")
                    for k in range(2)]
        for k in range(2):
            nc.vector.memset(h2hi[k][:].bitcast(F32) if PREC2 == 'f32r'
                             else h2hi[k][:], 0.0)
            if h2lo is not None:
                nc.vector.memset(h2lo[k][:], 0.0)

        # ---------------- phase 1+2: conv1 (fused strips) + conv2
        with tc.tile_pool(name="xt", bufs=1) as xtp, \
             tc.tile_pool(name="h1", bufs=2) as h1p, \
             tc.tile_pool(name="ps1", bufs=3, space="PSUM") as ps1, \
             tc.tile_pool(name="ps2", bufs=3, space="PSUM") as ps2:

            prev_h1 = None
            for s in range(8):
                base = 32 * s - 2                      # h1 row of tile row 0
                # strips >0 copy the 3-row halo from the previous strip tile
                # instead of recomputing it
                y_lo = 0 if s == 0 else 32 * s + 1
                y_hi = min(256, base + 35)
                R = y_hi - y_lo
                t_lo = y_lo - base                     # first valid tile row

                xt = xtp.tile([96, 35, 256], BF16, tag="xt")
                if s == 0:
                    # row-split so conv1's first chunks start after ~8 rows land
                    nc.sync.dma_start(xt[0:48, :8, :], xim[0:48, y_lo:y_lo + 8, :])
                    nc.scalar.dma_start(xt[48:96, :8, :], xim[48:96, y_lo:y_lo + 8, :])
                    nc.sync.dma_start(xt[0:48, 8:R, :], xim[0:48, y_lo + 8:y_hi, :])
                    nc.scalar.dma_start(xt[48:96, 8:R, :], xim[48:96, y_lo + 8:y_hi, :])
                else:
                    nc.sync.dma_start(xt[0:48, :R, :], xim[0:48, y_lo:y_hi, :])
                    nc.scalar.dma_start(xt[48:96, :R, :], xim[48:96, y_lo:y_hi, :])

                h1t = [h1p.tile([128, 35, 260], F32R, tag="h1r", name="h1r")]
                for t in h1t:
                    tv = t.bitcast(F32)
                    nc.vector.memset(tv[:, :, 0:2], 0.0)
                    nc.vector.memset(tv[:, :, 258:260], 0.0)
                    if s == 0:
                        nc.vector.memset(tv[:, 0:t_lo, :], 0.0)
                    else:
                        # halo rows 32s-2..32s+1 = prev tile rows 32..35
                        nc.vector.tensor_copy(t[:, 0:3, :], prev_h1[:, 32:35, :])
                    if t_lo + R < 35:
                        nc.vector.memset(tv[:, t_lo + R:35, :], 0.0)
                prev_h1 = h1t[0]

                # conv1 matmuls: chunks of 2 h1 rows (N=512)
                r = 0
                while r < R:
                    nrow = min(2, R - r)
                    N = nrow * 256
                    ps = ps1.tile([128, 512], F32, tag="c1")
                    rhs = xt[:, r:r + nrow, :]
                    nc.tensor.matmul(ps[:, :N], w1a_s[:], rhs, start=True, stop=False)
                    nc.tensor.matmul(ps[:, :N], w1b_s[:], xt[0:48, r:r + nrow, :],
                                     start=False, stop=True)
                    psv = ps[:, :N].rearrange("p (a b) -> p a b", a=nrow)
                    if PREC2 == 'f32r':
                        dst = h1t[0][:, t_lo + r:t_lo + r + nrow, 2:258]
                        nc.scalar.activation(dst, psv, RELU, bias=b1_s[:])
                    else:
                        raise NotImplementedError("hilo conv2 removed")
                    r += nrow

                # conv2 for this strip: h2 rows [8s, 8s+8)
                for h in range(2):
                    ps = ps2.tile([128, 512], F32, tag="c2")
                    first = True
                    for t in range(49):
                        dy, dx = divmod(t, 7)
                        if PREC2 == 'f32r':
                            rhs = h1t[0][:, dy:dy + 29:4, dx:dx + 253:4]
                            nc.tensor.matmul(ps[:], w2r_s[:, t, 128 * h:128 * h + 128],
                                             rhs, start=first, stop=(t == 48))
                            first = False
                        else:
                            rhs_hi = h1t[0][:, dy:dy + 29:4, dx:dx + 253:4]
                            rhs_lo = h1t[1][:, dy:dy + 29:4, dx:dx + 253:4]
                            lt_hi = w2hi_s[:, t, 128 * h:128 * h + 128]
                            lt_lo = w2lo_s[:, t, 128 * h:128 * h + 128]
                            nc.tensor.matmul(ps[:], lt_hi, rhs_hi, start=first,
                                             stop=False)
                            first = False
                            nc.tensor.matmul(ps[:], lt_hi, rhs_lo, start=False,
                                             stop=False)
                            nc.tensor.matmul(ps[:], lt_lo, rhs_hi, start=False,
                                             stop=(t == 48))
                    psv = ps[:].rearrange("p (a b) -> p a b", a=8)
                    dhi = h2hi[h][:, 2 + 8 * s:10 + 8 * s, 2:66]
                    if PREC2 == 'f32r':
                        nc.scalar.activation(dhi, psv, RELU, bias=b2_s[:, h:h + 1])
                    else:
                        raise NotImplementedError("hilo conv2 removed")

        ctx12.close()   # free conv2 weight pool

        # ---------------- phase 3: conv3
        h3p = ctx.enter_context(tc.tile_pool(name="h3p", bufs=1))
        h3 = [h3p.tile([128, 18, 18], F32, tag=f"h3_{k}", name=f"h3_{k}")
              for k in range(2)]
        for k in range(2):
            nc.vector.memset(h3[k][:], 0.0)

        b3_s = pers.tile([128, 2], F32, tag="b3")
        nc.gpsimd.dma_start(b3_s[:], b3[:])

        # prefetch ALL phase-4/5 weights during conv3 (one merged DMA)
        p4a = ctx.enter_context(tc.tile_pool(name="p4a", bufs=1))
        wtail_s = p4a.tile([128, 22016], F32, tag="wtail")
        nc.scalar.dma_start(wtail_s[:, 0:4608], wtail[:, 0:4608])
        nc.scalar.dma_start(wtail_s[:, 4608:12800], wtail[:, 4608:12800])
        nc.sync.dma_start(wtail_s[:, 12800:22016], wtail[:, 12800:22016])
        w4_s = wtail_s[:, 0:4608].rearrange("p (a b c) -> p a b c", a=2, b=9)
        rw1_s = [wtail_s[:, 4608:10752].rearrange("p (a b c) -> p a b c", a=2, b=3),
                 wtail_s[:, 12800:18944].rearrange("p (a b c) -> p a b c", a=2, b=3)]
        rw2_s = [wtail_s[:, 10752:12800].rearrange("p (a b) -> p a b", a=8),
                 wtail_s[:, 18944:20992].rearrange("p (a b) -> p a b", a=8)]
        embT_s = wtail_s[:, 20992:22016].rearrange("p (a b) -> p a b", a=2)

        # phase-4 pools created before conv3's scoped pools so conv4's
        # eviction targets and PSUM banks don't wait on conv3 teardown
        p4 = ctx.enter_context(tc.tile_pool(name="p4", bufs=1))
        ps4 = ctx.enter_context(tc.tile_pool(name="ps4", bufs=2, space="PSUM"))
        b4_s = pers.tile([128, 2], F32, tag="b4")
        nc.gpsimd.dma_start(b4_s[:], b4[:])
        h4 = [p4.tile([128, 8, 10], F32, tag=f"h4_{k}", name=f"h4_{k}")
              for k in range(2)]
        hrel = [p4.tile([128, 8, 10], F32, tag=f"hrel_{k}", name=f"hrel_{k}")
                for k in range(2)]
        for k in range(2):
            nc.vector.memset(h4[k][:], 0.0)
            nc.vector.memset(hrel[k][:], 0.0)

        TG = [(0, 13), (13, 13), (26, 13), (39, 10)]
        with tc.tile_pool(name="w3ring", bufs=3) as w3ring, \
             tc.tile_pool(name="w3tmp", bufs=2) as w3tmp, \
             tc.tile_pool(name="ps3", bufs=2, space="PSUM") as ps3:
            assert PREC3 == 'f32r', "only f32r conv3 implemented in this version"
            psh = [ps3.tile([128, 256], F32, tag=f"c3_{h}", name=f"c3_{h}")
                   for h in range(2)]
            first = True
            for kc in range(2):
                for (t0, tn) in TG:
                    w3f_t = w3tmp.tile([128, 13, 256], F32, tag="w3f", name="w3f")
                    nc.gpsimd.dma_start(w3f_t[:, :tn, :], w3f[:, kc, t0:t0 + tn])
                    w3r_t = w3ring.tile([128, 13, 256], F32R, tag="w3r", name="w3r")
                    nc.vector.tensor_copy(w3r_t[:, :tn, :], w3f_t[:, :tn, :])
                    for ti in range(tn):
                        t = t0 + ti
                        dy, dx = divmod(t, 7)
                        last = (kc == 1 and t == 48)
                        rhs = h2hi[kc][:, dy:dy + 61:4, dx:dx + 61:4]
                        for h in range(2):
                            nc.tensor.matmul(
                                psh[h][:], w3r_t[:, ti, 128 * h:128 * h + 128],
                                rhs, start=first, stop=last)
                        first = False
            for h in range(2):
                psv = psh[h][:].rearrange("p (a b) -> p a b", a=16)
                nc.scalar.activation(h3[h][:, 1:17, 1:17], psv, RELU,
                                     bias=b3_s[:, h:h + 1])

        # ---------------- phase 4: conv4 + residual blocks (fp32)

        for h in range(2):
            ps = ps4.tile([128, 64], F32, tag="c4")
            first = True
            for kc in range(2):
                for t in range(9):
                    dy, dx = divmod(t, 3)
                    rhs = h3[kc][:, dy:dy + 15:2, dx:dx + 15:2]
                    nc.tensor.matmul(ps[:], w4_s[:, kc, t, 128 * h:128 * h + 128],
                                     rhs, start=first,
                                     stop=(kc == 1 and t == 8))
                    first = False
            psv = ps[:].rearrange("p (a b) -> p a b", a=8)
            nc.scalar.activation(h4[h][:, :, 1:9], psv, RELU, bias=b4_s[:, h:h + 1])

        t1r = p4.tile([128, 8, 64], F32, tag="t1r")
        for b in range(2):
            src = h4 if b == 0 else hrel
            # t1 = relu(conv1x3(src))  [1024 ch in 8 chunks]
            for m in range(8):
                ps = ps4.tile([128, 64], F32, tag="c4")
                first = True
                for kc in range(2):
                    for dx in range(3):
                        rhs = src[kc][:, :, dx:dx + 8]
                        nc.tensor.matmul(
                            ps[:], rw1_s[b][:, kc, dx, 128 * m:128 * m + 128],
                            rhs, start=first, stop=(kc == 1 and dx == 2))
                        first = False
                nc.scalar.activation(t1r[:, m, :], ps[:], RELU)
            # t2 = conv1x1(t1); h4 += t2
            for h in range(2):
                ps = ps4.tile([128, 64], F32, tag="c4")
                for kc in range(8):
                    nc.tensor.matmul(ps[:], rw2_s[b][:, kc, 128 * h:128 * h + 128],
                                     t1r[:, kc, :], start=(kc == 0), stop=(kc == 7))
                psv = ps[:].rearrange("p (a b) -> p a b", a=8)
                h4s = h4[h][:, :, 1:9]
                nc.vector.tensor_add(h4s, h4s, psv)
                if b == 0:
                    nc.scalar.activation(hrel[h][:, :, 1:9], h4s, RELU)

        # ---------------- phase 5: lat + VQ cross term
        lat = [p4.tile([128, 64], F32, tag=f"lat{k}", name=f"lat{k}")
               for k in range(2)]
        for h in range(2):
            nc.scalar.activation(
                lat[h][:].rearrange("p (a b) -> p a b", a=8),
                h4[h][:, :, 1:9], RELU)
            nc.sync.dma_start(out_lat[h], lat[h][:])

        psC = ps4.tile([64, 512], F32, tag="psC")
        for kc in range(2):
            nc.tensor.matmul(psC[:], lat[kc][:], embT_s[:, kc, :],
                             start=(kc == 0), stop=(kc == 1))
        twoC = p4.tile([64, 512], F32, tag="twoC")
        nc.scalar.activation(twoC[:], psC[:], COPY, scale=2.0)
        nc.sync.dma_start(out_twoC[:], twoC[:])

        if debug:
            for k in range(2):
                nc.sync.dma_start(di['dbg_h2hi'][k], h2hi[k][:])
                if h2lo is not None:
                    nc.sync.dma_start(di['dbg_h2lo'][k], h2lo[k][:])
                nc.sync.dma_start(di['dbg_h3'][k], h3[k][:])
                nc.sync.dma_start(di['dbg_h4'][k], h4[k][:])

    nc.compile()
    _CACHE[key] = (nc, di)
    return _CACHE[key]


# ---------------------------------------------------------------- kernel()

def kernel(x, w_in, b_in, w_h1, b_h1, w_h2, b_h2, w_h3, b_h3,
           r0_w1, r0_w2, r1_w1, r1_w2, emb, debug=False, _res_out=None):
    x = np.asarray(x, np.float32)
    emb = np.asarray(emb, np.float32)
    shared = _prep_shared(np.asarray(w_in, np.float32), np.asarray(b_in, np.float32),
                          np.asarray(w_h1, np.float32), np.asarray(b_h1, np.float32),
                          np.asarray(w_h2, np.float32), np.asarray(b_h2, np.float32),
                          np.asarray(w_h3, np.float32), np.asarray(b_h3, np.float32),
                          np.asarray(r0_w1, np.float32), np.asarray(r0_w2, np.float32),
                          np.asarray(r1_w1, np.float32), np.asarray(r1_w2, np.float32),
                          emb)
    nc, di = _build(debug=debug)

    in_maps = []
    for b in range(8):
        m = dict(shared)
        m['xim'] = _prep_x(x[b])
        in_maps.append(m)

    res = run_bass_kernel_spmd(nc, in_maps, core_ids=list(range(8)))
    if _res_out is not None:
        _res_out.append(res)

    # host: replicate reference's f32 distance + argmin, gather, losses
    lats, idxs = [], []
    B = (emb * emb).sum(axis=1)                       # f32 [512]
    for b in range(8):
        r = res.results[b]
        lat = r['out_lat'].transpose(2, 0, 1).reshape(64, 256)  # [px, D]
        A = (lat * lat).sum(axis=1, keepdims=True)    # f32 [64,1]
        twoC = r['out_twoC']                          # [64,512] f32
        dist = (A + B) - twoC                         # f32, same assoc as reference
        idx = dist.argmin(axis=1)
        lats.append(lat)
        idxs.append(idx)

    lat_all = np.concatenate(lats, axis=0)            # [512, 256]
    idx_all = np.concatenate(idxs, axis=0)            # [512]
    quant = emb[idx_all]                              # [512, 256]

    quant_st = quant.reshape(8, 8, 8, 256).transpose(0, 3, 1, 2).copy()
    diff = quant.astype(np.float64) - lat_all.astype(np.float64)
    e_loss = np.float32((diff * diff).mean())
    loss = np.float32(e_loss + BETA * e_loss)
    counts = np.bincount(idx_all, minlength=512)
    probs = (counts / 512.0).astype(np.float32)
    plog = probs * np.log(probs + np.float32(EPS), dtype=np.float32)
    perplexity = np.float32(np.exp(-plog.sum(dtype=np.float32)))

    return quant_st, loss, perplexity
